# revision 13
# baseline (speedup 1.0000x reference)
"""GAT (graph attention) + global mean pool kernel for 8 Trainium2 NeuronCores.

Strategy (dst-sharded edges, no collectives):
  - Host: add self-loops, assign destination nodes to 8*49 blocks of <=128
    nodes (greedy bin-packing balancing, per block, the edge counts coming
    from each half of the source-id space), remap edges to
    (core, block, tile, lane), compute per-edge attention logits
    e_raw = a_src[src] + a_dst[dst] (a tiny x @ Wa projection), and build
    per-(tile,lane) one-hot selection matrices (lane -> dst row).
  - Device (SPMD, 8 cores), per core, streaming over edge tiles of 128:
      * dma_gather(transpose=True) fetches x[src] columns straight into
        SBUF as a [128 feat, edges] lhsT (int16 indices; the node table is
        split in two halves so indices fit in int16),
      * h_src = x_src @ W on TensorE (W column-permuted channel-major so the
        later broadcast multiply runs in the DVE 2x perf mode),
      * w = exp(max(e_raw, 0.2*e_raw)) (leaky relu on DVE, exp on ScalarE),
      * msg = [w * h_src | w] (ScalarE copies h PSUM->SBUF, DVE multiplies),
      * per-dst-block PSUM accumulation via selection-matrix matmuls,
      * block finalize: numer/denom, mean over heads, +bias, relu, and a
        one-hot matmul accumulating per-graph pooled sums.
  - Host: sum the 8 partial pooled tensors, divide by graph sizes, apply
    the final linear layer.

The softmax skips the max-subtraction pass: logits are O(1) for this
problem (asserted on host), so exp() cannot overflow and the result is
mathematically identical to the reference's max-shifted softmax.
"""

import sys

sys.path.insert(0, "/opt/trn_rl_repo/concourse")
sys.path.insert(0, "/opt/trn_rl_repo")

from dataclasses import dataclass

import numpy as np
import ml_dtypes

# ---- problem constants (hardcoded per contest rules) ----
N, E_RAW, IN, H, C, G = 50000, 800000, 128, 8, 32, 64
HC = H * C          # 256
NEG = 0.2           # leaky relu slope
P = 128
NCORES = 8
NBLK = 49           # dst blocks per core
TOTBLK = NCORES * NBLK
NT_TILES = 392      # node tiles across both table halves (392*128 >= N)
NTP = NT_TILES * P
HALF_TILES = NT_TILES // 2      # 196
HALF_ROWS = HALF_TILES * P      # 25088 (< 2^15, fits int16 indexing)
GBLK = 3            # blocks per dma_gather / sel-stream chunk

bf16 = ml_dtypes.bfloat16


@dataclass(frozen=True)
class PlanKey:
    ka: int
    kb: int


_COMPILED = {}


# --------------------------------------------------------------------------
# host-side preprocessing
# --------------------------------------------------------------------------

def _pack_blocks(degA, degB):
    """Greedy 2D bin-packing: nodes -> TOTBLK blocks of <=128 nodes,
    balancing per-block edge sums from each source half."""
    tot = degA + degB
    order = np.argsort(-tot, kind="stable")
    sumA = np.zeros(TOTBLK, np.int64)
    sumB = np.zeros(TOTBLK, np.int64)
    cnt = np.zeros(TOTBLK, np.int64)
    node_block = np.empty(N, np.int32)
    node_row = np.empty(N, np.int32)
    BIG = 1 << 40
    for n in order:
        dA = degA[n]
        dB = degB[n]
        score = np.maximum(sumA + dA, sumB + dB) + (cnt >= P) * BIG
        b = int(np.argmin(score))
        node_block[n] = b
        node_row[n] = cnt[b]
        cnt[b] += 1
        sumA[b] += dA
        sumB[b] += dB
    assert cnt.max() <= P
    ka = int(np.ceil(sumA.max() / P))
    kb = int(np.ceil(sumB.max() / P))
    return node_block, node_row, ka, kb


def _wrap_idx16(flat):
    """Flat index list (len % 128 == 0) -> [128, len//16] int16 wrapped in 16
    partitions and replicated for the 8 gpsimd cores."""
    a = flat.reshape(-1, 16).T.astype(np.int16)
    return np.tile(a, (8, 1))


def preprocess(x, edge_index, batch, W, att_src, att_dst, bias):
    x = np.asarray(x, np.float32)
    ei = np.asarray(edge_index)
    batch = np.asarray(batch).astype(np.int64)
    W = np.asarray(W, np.float32)
    att_src = np.asarray(att_src, np.float32)
    att_dst = np.asarray(att_dst, np.float32)
    bias = np.asarray(bias, np.float32)

    loops = np.arange(N, dtype=np.int64)
    src = np.concatenate([np.asarray(ei[0], np.int64), loops])
    dst = np.concatenate([np.asarray(ei[1], np.int64), loops])
    ET = src.shape[0]

    half = (src >= HALF_ROWS).astype(np.int64)   # 0 = table A, 1 = table B
    degA = np.bincount(dst[half == 0], minlength=N)
    degB = np.bincount(dst[half == 1], minlength=N)
    node_block, node_row, ka, kb = _pack_blocks(degA, degB)

    # per-edge attention logits (host: ~0.2% of the problem flops)
    Wa_s = (W.reshape(IN, H, C) * att_src[None]).sum(-1)   # [IN, H]
    Wa_d = (W.reshape(IN, H, C) * att_dst[None]).sum(-1)
    a_src = x @ Wa_s
    a_dst = x @ Wa_d
    er = a_src[src] + a_dst[dst]                            # [ET, H] f32
    assert np.abs(er).max() < 60.0, "logits too large for exp without max-shift"

    TA = NBLK * ka
    TB = NBLK * kb
    T = TA + TB

    # group edges by (global block, half); position within group -> tile/lane
    key = node_block[dst] * 2 + half
    eorder = np.argsort(key, kind="stable")
    kcnt = np.bincount(key, minlength=TOTBLK * 2)
    starts = np.concatenate([[0], np.cumsum(kcnt)])[:-1]
    pos = np.arange(ET) - starts[key[eorder]]

    es, ed, eh = src[eorder], dst[eorder], half[eorder]
    eb = node_block[ed]
    core = eb // NBLK
    b_in_core = eb % NBLK
    khalf = np.where(eh == 0, ka, kb)
    assert (pos < khalf * P).all()
    tile_local = pos // P
    lane = pos % P
    region0 = np.where(eh == 0, 0, TA)
    t = region0 + b_in_core * khalf + tile_local

    srcrel = np.where(eh == 0, es, es - HALF_ROWS).astype(np.int16)
    flatidx = np.zeros((NCORES, T * P), np.int16)       # pad -> row 0
    eraw = np.full((NCORES, P, T, H), -1.0e4, np.float32)
    flatidx[core, t * P + lane] = srcrel
    eraw[core, lane, t] = er[eorder]

    # one-hot selection matrices (lane -> dst row), bf16, per tile
    drow = node_row[ed].astype(np.int64)
    selA = np.zeros((NCORES, P, TA * P), bf16)
    selB = np.zeros((NCORES, P, TB * P), bf16)
    mA = eh == 0
    selA[core[mA], lane[mA], t[mA] * P + drow[mA]] = 1.0
    mB = ~mA
    selB[core[mB], lane[mB], (t[mB] - TA) * P + drow[mB]] = 1.0

    # wrapped int16 index arrays, one contiguous column range per gather op
    ngA = -(-NBLK // GBLK)
    idxA = np.zeros((NCORES, P, TA * P // 16), np.int16)
    idxB = np.zeros((NCORES, P, TB * P // 16), np.int16)
    for c in range(NCORES):
        fi = flatidx[c]
        for g in range(ngA):
            nb = min(GBLK, NBLK - g * GBLK)
            a0 = g * GBLK * ka * P
            idxA[c][:, a0 // 16:(a0 + nb * ka * P) // 16] = _wrap_idx16(
                fi[a0:a0 + nb * ka * P])
            b0 = g * GBLK * kb * P
            idxB[c][:, b0 // 16:(b0 + nb * kb * P) // 16] = _wrap_idx16(
                fi[TA * P + b0:TA * P + b0 + nb * kb * P])

    # per-(block,row) graph ids; pads -> G (never matches iota 0..G-1)
    bidx = np.full((NCORES, P, NBLK), float(G), np.float32)
    bidx[node_block // NBLK, node_row, node_block % NBLK] = (
        batch.astype(np.float32))

    # node table, split in halves, bf16 rows of 256B
    xpad = np.zeros((NTP, IN), np.float32)
    xpad[:N] = x
    xbf = xpad.astype(bf16)
    # W with channel-major columns: Wp[:, c*H + h] = W[:, h*C + c]
    Wp = np.ascontiguousarray(
        W.reshape(IN, H, C).transpose(0, 2, 1).reshape(IN, HC)).astype(bf16)

    shared = dict(
        xbfA=np.ascontiguousarray(xbf[:HALF_ROWS]),
        xbfB=np.ascontiguousarray(xbf[HALF_ROWS:]),
        wt=Wp,
        iotaf=np.tile(np.arange(P, dtype=np.float32), (P, 1)),
        biasr=np.tile(bias.astype(np.float32), (P, 1)),
    )
    per_core = []
    for c in range(NCORES):
        m = dict(shared)
        m.update(
            idxA=idxA[c], idxB=idxB[c],
            selA=selA[c], selB=selB[c],
            eraw=eraw[c].reshape(P, T * H),
            bidx=bidx[c],
        )
        per_core.append(m)

    cntg = np.bincount(batch, minlength=G).astype(np.float32)
    return per_core, PlanKey(ka, kb), cntg


# --------------------------------------------------------------------------
# device program
# --------------------------------------------------------------------------

def build_program(plan: PlanKey):
    from concourse import bacc
    import concourse.mybir as mybir
    import concourse.tile as tile

    ka, kb = plan.ka, plan.kb
    TA = NBLK * ka
    TB = NBLK * kb
    T = TA + TB
    dt = mybir.dt
    f32, bft, i16 = dt.float32, dt.bfloat16, dt.int16
    AX = mybir.AxisListType
    OP = mybir.AluOpType
    ACT = mybir.ActivationFunctionType

    nc = bacc.Bacc("TRN2", debug=False)
    xbfA = nc.dram_tensor("xbfA", [HALF_ROWS, IN], bft, kind="ExternalInput")
    xbfB = nc.dram_tensor("xbfB", [HALF_ROWS, IN], bft, kind="ExternalInput")
    wt = nc.dram_tensor("wt", [IN, HC], bft, kind="ExternalInput")
    iotaf = nc.dram_tensor("iotaf", [P, P], f32, kind="ExternalInput")
    biasr = nc.dram_tensor("biasr", [P, C], f32, kind="ExternalInput")
    idxA = nc.dram_tensor("idxA", [P, TA * P // 16], i16, kind="ExternalInput")
    idxB = nc.dram_tensor("idxB", [P, TB * P // 16], i16, kind="ExternalInput")
    selA = nc.dram_tensor("selA", [P, TA * P], bft, kind="ExternalInput")
    selB = nc.dram_tensor("selB", [P, TB * P], bft, kind="ExternalInput")
    eraw = nc.dram_tensor("eraw", [P, T * H], f32, kind="ExternalInput")
    bidx = nc.dram_tensor("bidx", [P, NBLK], f32, kind="ExternalInput")
    pout = nc.dram_tensor("pout", [G, C], f32, kind="ExternalOutput")

    with tile.TileContext(nc) as tc:
        with (
            tc.tile_pool(name="const", bufs=1) as cp,
            tc.tile_pool(name="ppool", bufs=1, space="PSUM") as ppl,
            tc.tile_pool(name="gp", bufs=2) as gp,
            tc.tile_pool(name="sp", bufs=2) as sp,
            tc.tile_pool(name="hp", bufs=2) as hp,
            tc.tile_pool(name="ep", bufs=2) as ep,
            tc.tile_pool(name="fp", bufs=2) as fp,
            tc.tile_pool(name="hps_p", bufs=3, space="PSUM") as hpp,
            tc.tile_pool(name="aggp", bufs=2, space="PSUM") as aggp,
        ):
            wt_sb = cp.tile([IN, HC], bft)
            nc.sync.dma_start(wt_sb[:], wt[:, :])
            iotaf_sb = cp.tile([P, P], f32)
            nc.sync.dma_start(iotaf_sb[:], iotaf[:, :])
            biasr_sb = cp.tile([P, C], f32)
            nc.sync.dma_start(biasr_sb[:], biasr[:, :])
            idxA_sb = cp.tile([P, TA * P // 16], i16)
            nc.sync.dma_start(idxA_sb[:], idxA[:, :])
            idxB_sb = cp.tile([P, TB * P // 16], i16)
            nc.sync.dma_start(idxB_sb[:], idxB[:, :])
            eraw_sb = cp.tile([P, T * H], f32)
            nc.sync.dma_start(eraw_sb[:], eraw[:, :])
            bidx_sb = cp.tile([P, NBLK], f32)
            nc.sync.dma_start(bidx_sb[:], bidx[:, :])

            pooled_ps = ppl.tile([G, C], f32, name="pooled_ps")

            ngA = -(-NBLK // GBLK)
            for g in range(ngA):
                nb = min(GBLK, NBLK - g * GBLK)
                bufs = {}
                for (nm, tbl_t, sel_t, idx_sb, kh) in (
                    ("A", xbfA, selA, idxA_sb, ka),
                    ("B", xbfB, selB, idxB_sb, kb),
                ):
                    L = nb * kh * P
                    xgT = gp.tile([P, GBLK * kh * P], bft, tag=f"xgT{nm}",
                                  name=f"xgT{nm}")
                    c0 = g * GBLK * kh * P // 16
                    nc.gpsimd.dma_gather(
                        out_ap=xgT[:, 0:L].rearrange("p (o l) -> p o l", o=1),
                        in_ap=tbl_t[:, :],
                        idxs_ap=idx_sb[:, c0:c0 + L // 16],
                        num_idxs=L,
                        num_idxs_reg=L,
                        elem_size=IN,
                        transpose=True,
                        single_packet=False,
                    )
                    sel_sb = sp.tile([P, GBLK * kh * P], bft, tag=f"sel{nm}",
                                     name=f"sel{nm}")
                    s0 = g * GBLK * kh * P
                    nc.sync.dma_start(sel_sb[:, 0:L], sel_t[:, s0:s0 + L])
                    bufs[nm] = (xgT, sel_sb)

                for bl in range(nb):
                    b = g * GBLK + bl
                    agg = aggp.tile([P, HC + H], f32, tag="agg", name="agg")
                    for (nm, kh, treg) in (("A", ka, 0), ("B", kb, TA)):
                        xgT, sel_sb = bufs[nm]
                        t0 = treg + b * kh
                        # w = exp(max(e, NEG*e))
                        lr = ep.tile([P, kh * H], f32, tag=f"lr{nm}",
                                     name=f"lr{nm}")
                        esl = eraw_sb[:, t0 * H:(t0 + kh) * H]
                        nc.vector.scalar_tensor_tensor(
                            out=lr[:], in0=esl, scalar=NEG, in1=esl,
                            op0=OP.mult, op1=OP.max)
                        w = ep.tile([P, kh * H], bft, tag=f"w{nm}",
                                    name=f"w{nm}")
                        nc.scalar.activation(w[:], lr[:], ACT.Exp)

                        # h_src = x_src @ Wp (channel-major), PSUM-bank pairs,
                        # ScalarE copies PSUM -> SBUF bf16
                        h_sb = hp.tile([P, kh, HC], bft, tag=f"hs{nm}",
                                       name=f"hs{nm}")
                        o0 = bl * kh * P
                        for j0 in range(0, kh, 2):
                            jn = min(2, kh - j0)
                            hps = hpp.tile([P, 2, HC], f32, tag="hps",
                                           name="hps")
                            for j in range(jn):
                                k = j0 + j
                                nc.tensor.matmul(
                                    hps[:, j, :],
                                    xgT[:, o0 + k * P:o0 + (k + 1) * P],
                                    wt_sb[:],
                                    start=True, stop=True)
                            nc.scalar.activation(
                                h_sb[:, j0:j0 + jn, :],
                                hps[:, 0:jn, :], ACT.Copy)

                        # msg = [w * h | w]   (channel-major: 2x DVE mode)
                        msg = ep.tile([P, kh, HC + H], bft, tag=f"msg{nm}",
                                      name=f"msg{nm}")
                        nc.vector.tensor_tensor(
                            out=msg[:, :, 0:HC].rearrange(
                                "p k (c h) -> p k c h", h=H),
                            in0=h_sb[:].rearrange("p k (c h) -> p k c h", h=H),
                            in1=w[:].rearrange("p (k h) -> p k h", h=H)
                            [:, :, None, :].to_broadcast([P, kh, C, H]),
                            op=OP.mult)
                        nc.vector.tensor_copy(
                            msg[:, :, HC:HC + H],
                            w[:].rearrange("p (k h) -> p k h", h=H))

                        first = nm == "A"
                        for k in range(kh):
                            nc.tensor.matmul(
                                agg[:],
                                sel_sb[:, (bl * kh + k) * P:
                                       (bl * kh + k + 1) * P],
                                msg[:, k, :],
                                start=(first and k == 0),
                                stop=(nm == "B" and k == kh - 1))

                    # ---- finalize block b (channel-major) ----
                    den = fp.tile([P, H], f32, tag="den", name="den")
                    nc.vector.tensor_scalar_add(
                        den[:], agg[:, HC:HC + H], 1e-16)
                    rec = fp.tile([P, H], f32, tag="rec", name="rec")
                    nc.vector.reciprocal(rec[:], den[:])
                    hidw = fp.tile([P, HC], f32, tag="hidw", name="hidw")
                    nc.vector.tensor_tensor(
                        out=hidw[:].rearrange("p (c h) -> p c h", h=H),
                        in0=agg[:, 0:HC].rearrange("p (c h) -> p c h", h=H),
                        in1=rec[:][:, None, :].to_broadcast([P, C, H]),
                        op=OP.mult)
                    red = fp.tile([P, C], f32, tag="red", name="red")
                    nc.vector.tensor_reduce(
                        red[:],
                        hidw[:].rearrange("p (c h) -> p c h", h=H),
                        axis=AX.X, op=OP.add)
                    pre = fp.tile([P, C], f32, tag="pre", name="pre")
                    nc.vector.scalar_tensor_tensor(
                        out=pre[:], in0=red[:], scalar=1.0 / H,
                        in1=biasr_sb[:], op0=OP.mult, op1=OP.add)
                    hid = fp.tile([P, C], bft, tag="hid", name="hid")
                    nc.scalar.activation(hid[:], pre[:], ACT.Relu)
                    bsel = fp.tile([P, G], bft, tag="bsel", name="bsel")
                    nc.vector.tensor_tensor(
                        out=bsel[:],
                        in0=bidx_sb[:, b][:, None].to_broadcast([P, G]),
                        in1=iotaf_sb[:, 0:G],
                        op=OP.is_equal)
                    nc.tensor.matmul(
                        pooled_ps[:], bsel[:], hid[:],
                        start=(b == 0), stop=(b == NBLK - 1))

            pooled_sb = cp.tile([G, C], f32)
            nc.vector.tensor_copy(pooled_sb[:], pooled_ps[:])
            nc.sync.dma_start(pout[:, :], pooled_sb[:])

    nc.compile()
    return nc


# --------------------------------------------------------------------------
# entry point
# --------------------------------------------------------------------------

def kernel(x, edge_index, batch, W, att_src, att_dst, bias, lin_w, lin_b):
    from concourse.bass_utils import run_bass_kernel_spmd

    per_core, plan, cntg = preprocess(
        x, edge_index, batch, W, att_src, att_dst, bias)

    if plan not in _COMPILED:
        _COMPILED[plan] = build_program(plan)
    nc = _COMPILED[plan]

    res = run_bass_kernel_spmd(nc, per_core, core_ids=list(range(NCORES)))
    pooled = np.zeros((G, C), np.float64)
    for r in res.results:
        pooled += r["pout"].astype(np.float64)
    pooled = (pooled / np.maximum(cntg, 1.0)[:, None]).astype(np.float32)
    x_t = pooled @ np.asarray(lin_w, np.float32) + np.asarray(lin_b, np.float32)
    return (x_t, pooled)
